# revision 12
# baseline (speedup 1.0000x reference)
"""Channel-attention module (CAM) kernel for Trainium2.

Reference computation (per batch b):
    a    = x[b].reshape(HW, C)                      # [4096, 512]
    aTa  = a.T @ a                                  # [512, 512]
    attn = softmax(aTa, axis=-1)
    y    = a @ attn                                 # [4096, 512]
    out[b] = gamma * y + x[b]

Sharding: data-parallel over batch B=16 across 8 NeuronCores (2 batches
per core), gamma replicated.  No collectives needed.

Per-core schedule (per batch):
  warmup  ~30 tiny matmuls while the first DMAs land, so the PE HAM clock
          gate is already at 8/8 when real work starts
  pass 1  aTa is symmetric: only diagonal+upper blocks are computed
          (rhs free dim 512/384/256/128 per column-block), lower blocks are
          mirrored via 6 PE transposes of the upper ones
  softmax rows of aTa (SBUF), folding gamma into the normalizer and adding
          the identity so pass 2 directly yields gamma*y + a = a @ (g*attn+I)
  tpose   a -> aT via PE transpose (128x128 blocks, 4 per PSUM bank),
          evacuated to SBUF by Vector/Scalar engines
  pass 2  y[k] (PSUM) += aT[cb][:,k128]-block @ attn'[cb]; copy to SBUF,
          DMA out.

All matmul operands are float32r (fp32 truncated to fp22 by the PE) so the
matmuls run at 1 cycle/row instead of fp32's 4.  The BIR verifier requires
every producer of an f32r matmul operand to emit f32r, so those tiles are
declared float32r and their writers (DMA / DVE / ACT copies) write f32r.
"""

import numpy as np

import concourse.bass as bass
import concourse.bacc as bacc
import concourse.mybir as mybir
import concourse.tile as tile
from concourse.bass_utils import run_bass_kernel_spmd
from concourse.masks import make_identity

B, H, W, C = 16, 64, 64, 512
HW = H * W                      # 4096
NCORES = 8
BPC = B // NCORES               # batches per core
NT = HW // 128                  # 32 row-chunks of a
CB = C // 128                   # 4 column-blocks of C
F32 = mybir.dt.float32
F32R = mybir.dt.float32r


def build_bass():
    nc = bacc.Bacc("TRN2", target_bir_lowering=False, debug=False)
    x = nc.dram_tensor("x", [BPC, HW, C], F32, kind="ExternalInput").ap()
    gamma = nc.dram_tensor("gamma", [1], F32, kind="ExternalInput").ap()
    out = nc.dram_tensor("out", [BPC, HW, C], F32, kind="ExternalOutput").ap()

    with tile.TileContext(nc) as tc:
        with (
            tc.tile_pool(name="singles", bufs=1) as singles,
            tc.tile_pool(name="a", bufs=36) as a_pool,
            tc.tile_pool(name="at", bufs=4) as at_pool,
            tc.tile_pool(name="atasb", bufs=6) as atasb_pool,
            tc.tile_pool(name="attn", bufs=6) as attn_pool,
            tc.tile_pool(name="stats", bufs=16) as stats_pool,
            tc.tile_pool(name="ostage", bufs=6) as out_pool,
            tc.tile_pool(name="psum", bufs=8, space="PSUM") as psum_pool,
        ):
            # -------- PE warmup: keep HAM busy while first DMAs land -------
            warm_f = singles.tile([128, 128], F32)
            nc.vector.memset(warm_f, 1.0)
            warm = singles.tile([128, 128], F32R)
            nc.vector.tensor_copy(warm, warm_f)
            wps = psum_pool.tile([128, 128], F32, tag="ps")
            for _ in range(10):
                nc.tensor.matmul(wps, warm, warm, start=True, stop=True)

            ident = singles.tile([128, 128], F32)
            make_identity(nc, ident)
            ident_r = singles.tile([128, 128], F32R)
            nc.vector.tensor_copy(ident_r, ident)
            gam = singles.tile([128, 1], F32)
            nc.gpsimd.dma_start(out=gam, in_=gamma.to_broadcast((128, 1)))

            for b in range(BPC):
                # ---------------- load a ----------------
                a_tiles = []
                for k in range(NT):
                    t = a_pool.tile([128, C], F32R, tag="a")
                    nc.sync.dma_start(
                        out=t, in_=x[b, k * 128:(k + 1) * 128, :].bitcast(F32R)
                    )
                    a_tiles.append(t)

                # ------- pass 1: aTa, diagonal + upper blocks only -------
                ata = [
                    psum_pool.tile([128, C], F32, tag="ps", name="ata")
                    for _ in range(CB)
                ]
                for k in range(NT):
                    ak = a_tiles[k]
                    for cb in range(CB):
                        nc.tensor.matmul(
                            ata[cb][:, cb * 128:C],
                            ak[:, cb * 128:(cb + 1) * 128],
                            ak[:, cb * 128:C],
                            start=(k == 0),
                            stop=(k == NT - 1),
                        )

                # ------- evacuate upper blocks + stage mirror sources -------
                asb = [
                    atasb_pool.tile([128, C], F32, tag="atasb", name="asb")
                    for _ in range(CB)
                ]
                for cb in range(CB):
                    if cb % 2 == 0:
                        nc.vector.tensor_copy(
                            asb[cb][:, cb * 128:C], ata[cb][:, cb * 128:C]
                        )
                    else:
                        nc.scalar.copy(
                            asb[cb][:, cb * 128:C], ata[cb][:, cb * 128:C]
                        )
                msrcs = {}
                for cb in range(CB):
                    for db in range(cb):
                        # stage upper block (db, cb) as f32r SBUF for the PE
                        msrc = atasb_pool.tile(
                            [128, 128], F32R, tag="msrc", name="msrc", bufs=8
                        )
                        if (cb + db) % 2 == 0:
                            nc.vector.tensor_copy(
                                msrc, ata[db][:, cb * 128:(cb + 1) * 128]
                            )
                        else:
                            nc.scalar.copy(
                                msrc, ata[db][:, cb * 128:(cb + 1) * 128]
                            )
                        msrcs[(cb, db)] = msrc

                # two a->aT transpose groups first: PE keeps busy while the
                # mirror sources are staged by DVE/ACT
                at_tiles = [
                    at_pool.tile([128, HW], F32R, tag="at", name="at")
                    for _ in range(CB)
                ]

                def tp_group(g, cb):
                    tp = psum_pool.tile(
                        [128, C], F32R, tag="ps", name="tp"
                    )
                    for j in range(4):
                        k = g * 4 + j
                        nc.tensor.transpose(
                            tp[:, j * 128:(j + 1) * 128],
                            a_tiles[k][:, cb * 128:(cb + 1) * 128],
                            ident_r,
                        )
                    dst = at_tiles[cb][:, g * 512:(g + 1) * 512]
                    if (g * CB + cb) % 3 == 0:
                        nc.vector.tensor_copy(dst, tp.bitcast(F32))
                    else:
                        nc.scalar.copy(dst, tp.bitcast(F32))

                for g in range(2):
                    for cb in range(CB):
                        tp_group(g, cb)

                # ------- mirror lower blocks: (cb,db) = (db,cb)^T -------
                for cb in range(CB):
                    for db in range(cb):
                        mir = psum_pool.tile(
                            [128, 128], F32R, tag="ps", name="mir"
                        )
                        nc.tensor.transpose(mir, msrcs[(cb, db)], ident_r)
                        if (cb + db) % 2 == 0:
                            nc.vector.tensor_copy(
                                asb[cb][:, db * 128:(db + 1) * 128],
                                mir.bitcast(F32),
                            )
                        else:
                            nc.scalar.copy(
                                asb[cb][:, db * 128:(db + 1) * 128],
                                mir.bitcast(F32),
                            )

                # ---------------- rest of a -> aT ----------------
                for g in range(2, NT // 4):
                    for cb in range(CB):
                        tp_group(g, cb)

                # ------------- softmax rows + gamma fold -------------
                attn_tiles = []
                for cb in range(CB):
                    negmax = stats_pool.tile([128, 1], F32, tag="st")
                    nc.vector.reduce_max(
                        negmax, asb[cb], axis=mybir.AxisListType.X, negate=True
                    )
                    rowsum = stats_pool.tile([128, 1], F32, tag="st")
                    nc.scalar.activation(
                        asb[cb],
                        asb[cb],
                        mybir.ActivationFunctionType.Exp,
                        bias=negmax,
                        scale=1.0,
                        accum_out=rowsum,
                    )
                    grec = stats_pool.tile([128, 1], F32, tag="st")
                    nc.vector.reciprocal(grec, rowsum)
                    # fold gamma into the row normalizer: attn' = gamma/rowsum * E
                    nc.vector.tensor_scalar_mul(grec, grec, gam)
                    nc.vector.tensor_scalar_mul(asb[cb], asb[cb], grec)
                    # + I on this tile's diagonal block so pass2 fuses the residual
                    nc.vector.tensor_add(
                        asb[cb][:, cb * 128:(cb + 1) * 128],
                        asb[cb][:, cb * 128:(cb + 1) * 128],
                        ident,
                    )
                    # rounded f32r copy for the pass-2 matmul
                    ar = attn_pool.tile([128, C], F32R, tag="attn")
                    nc.vector.tensor_copy(ar, asb[cb])
                    attn_tiles.append(ar)

                # ------------ pass 2: out = a @ (g*attn + I) ------------
                for k in range(NT):
                    yp = psum_pool.tile([128, C], F32, tag="ps")
                    for cb in range(CB):
                        nc.tensor.matmul(
                            yp,
                            at_tiles[cb][:, k * 128:(k + 1) * 128],
                            attn_tiles[cb],
                            start=(cb == 0),
                            stop=(cb == CB - 1),
                        )
                    o = out_pool.tile([128, C], F32, tag="o")
                    if k % 2 == 0:
                        nc.vector.tensor_copy(o, yp)
                    else:
                        nc.scalar.copy(o, yp)
                    nc.sync.dma_start(out=out[b, k * 128:(k + 1) * 128, :], in_=o)

    nc.compile()
    return nc


_NC_CACHE = None


def _get_nc():
    global _NC_CACHE
    if _NC_CACHE is None:
        _NC_CACHE = build_bass()
    return _NC_CACHE


def make_in_maps(x: np.ndarray, gamma: np.ndarray):
    x = np.ascontiguousarray(np.asarray(x, dtype=np.float32)).reshape(B, HW, C)
    gamma = np.ascontiguousarray(np.asarray(gamma, dtype=np.float32)).reshape(1)
    return [
        {"x": x[i * BPC:(i + 1) * BPC], "gamma": gamma} for i in range(NCORES)
    ]


def kernel(x: np.ndarray, gamma: np.ndarray, _trace: bool = False, _tmpdir=None):
    nc = _get_nc()
    in_maps = make_in_maps(x, gamma)
    res = run_bass_kernel_spmd(
        nc, in_maps, list(range(NCORES)), trace=_trace, tmpdir=_tmpdir
    )
    outs = [np.asarray(res.results[i]["out"]) for i in range(NCORES)]
    full = np.concatenate(outs, axis=0).reshape(B, H, W, C)
    if _trace:
        return full, res
    return full


# revision 14
# speedup vs baseline: 1.0381x; 1.0381x over previous
"""Channel-attention module (CAM) kernel for Trainium2.

Reference computation (per batch b):
    a    = x[b].reshape(HW, C)                      # [4096, 512]
    aTa  = a.T @ a                                  # [512, 512]
    attn = softmax(aTa, axis=-1)
    y    = a @ attn                                 # [4096, 512]
    out[b] = gamma * y + x[b]

Sharding: data-parallel over batch B=16 across 8 NeuronCores (2 batches
per core), gamma replicated.  No collectives needed.

Per-core schedule (per batch):
  warmup  ~30 tiny matmuls while the first DMAs land, so the PE HAM clock
          gate is already at 8/8 when real work starts
  pass 1  aTa is symmetric: only diagonal+upper blocks are computed
          (rhs free dim 512/384/256/128 per column-block), lower blocks are
          mirrored via 6 PE transposes of the upper ones
  softmax rows of aTa (SBUF), folding gamma into the normalizer and adding
          the identity so pass 2 directly yields gamma*y + a = a @ (g*attn+I)
  tpose   a -> aT via PE transpose (128x128 blocks, 4 per PSUM bank),
          evacuated to SBUF by Vector/Scalar engines
  pass 2  y[k] (PSUM) += aT[cb][:,k128]-block @ attn'[cb]; copy to SBUF,
          DMA out.

All matmul operands are float32r (fp32 truncated to fp22 by the PE) so the
matmuls run at 1 cycle/row instead of fp32's 4.  The BIR verifier requires
every producer of an f32r matmul operand to emit f32r, so those tiles are
declared float32r and their writers (DMA / DVE / ACT copies) write f32r.
"""

import numpy as np

import concourse.bass as bass
import concourse.bacc as bacc
import concourse.mybir as mybir
import concourse.tile as tile
from concourse.bass_utils import run_bass_kernel_spmd
from concourse.masks import make_identity

B, H, W, C = 16, 64, 64, 512
HW = H * W                      # 4096
NCORES = 8
BPC = B // NCORES               # batches per core
NT = HW // 128                  # 32 row-chunks of a
CB = C // 128                   # 4 column-blocks of C
F32 = mybir.dt.float32
F32R = mybir.dt.float32r


def build_bass():
    nc = bacc.Bacc("TRN2", target_bir_lowering=False, debug=False)
    x = nc.dram_tensor("x", [BPC, HW, C], F32, kind="ExternalInput").ap()
    gamma = nc.dram_tensor("gamma", [1], F32, kind="ExternalInput").ap()
    out = nc.dram_tensor("out", [BPC, HW, C], F32, kind="ExternalOutput").ap()

    with tile.TileContext(nc) as tc:
        with (
            tc.tile_pool(name="singles", bufs=1) as singles,
            tc.tile_pool(name="a", bufs=36) as a_pool,
            tc.tile_pool(name="at", bufs=4) as at_pool,
            tc.tile_pool(name="atasb", bufs=6) as atasb_pool,
            tc.tile_pool(name="attn", bufs=6) as attn_pool,
            tc.tile_pool(name="stats", bufs=16) as stats_pool,
            tc.tile_pool(name="ostage", bufs=6) as out_pool,
            tc.tile_pool(name="psum", bufs=8, space="PSUM") as psum_pool,
        ):
            ident = singles.tile([128, 128], F32)
            make_identity(nc, ident)
            ident_r = singles.tile([128, 128], F32R)
            nc.vector.tensor_copy(ident_r, ident)
            gam = singles.tile([128, 1], F32)
            nc.gpsimd.dma_start(out=gam, in_=gamma.to_broadcast((128, 1)))

            for b in range(BPC):
                # ---------------- load a ----------------
                a_tiles = []
                for k in range(NT):
                    t = a_pool.tile([128, C], F32R, tag="a")
                    nc.sync.dma_start(
                        out=t, in_=x[b, k * 128:(k + 1) * 128, :].bitcast(F32R)
                    )
                    a_tiles.append(t)

                # ------- pass 1: aTa, diagonal + upper blocks only -------
                ata = [
                    psum_pool.tile([128, C], F32, tag="ps", name="ata")
                    for _ in range(CB)
                ]
                for k in range(NT):
                    ak = a_tiles[k]
                    for cb in range(CB):
                        nc.tensor.matmul(
                            ata[cb][:, cb * 128:C],
                            ak[:, cb * 128:(cb + 1) * 128],
                            ak[:, cb * 128:C],
                            start=(k == 0),
                            stop=(k == NT - 1),
                        )

                # ------- evacuate upper blocks + stage mirror sources -------
                asb = [
                    atasb_pool.tile([128, C], F32, tag="atasb", name="asb")
                    for _ in range(CB)
                ]
                for cb in range(CB):
                    if cb % 2 == 0:
                        nc.vector.tensor_copy(
                            asb[cb][:, cb * 128:C], ata[cb][:, cb * 128:C]
                        )
                    else:
                        nc.scalar.copy(
                            asb[cb][:, cb * 128:C], ata[cb][:, cb * 128:C]
                        )
                msrcs = {}
                for cb in range(CB):
                    for db in range(cb):
                        # stage upper block (db, cb) as f32r SBUF for the PE
                        msrc = atasb_pool.tile(
                            [128, 128], F32R, tag="msrc", name="msrc", bufs=8
                        )
                        if (cb + db) % 2 == 0:
                            nc.vector.tensor_copy(
                                msrc, ata[db][:, cb * 128:(cb + 1) * 128]
                            )
                        else:
                            nc.scalar.copy(
                                msrc, ata[db][:, cb * 128:(cb + 1) * 128]
                            )
                        msrcs[(cb, db)] = msrc

                # two a->aT transpose groups first: PE keeps busy while the
                # mirror sources are staged by DVE/ACT
                at_tiles = [
                    at_pool.tile([128, HW], F32R, tag="at", name="at")
                    for _ in range(CB)
                ]

                def tp_group(g, cb):
                    tp = psum_pool.tile(
                        [128, C], F32R, tag="ps", name="tp"
                    )
                    for j in range(4):
                        k = g * 4 + j
                        nc.tensor.transpose(
                            tp[:, j * 128:(j + 1) * 128],
                            a_tiles[k][:, cb * 128:(cb + 1) * 128],
                            ident_r,
                        )
                    dst = at_tiles[cb][:, g * 512:(g + 1) * 512]
                    if (g * CB + cb) % 3 == 0:
                        nc.vector.tensor_copy(dst, tp.bitcast(F32))
                    else:
                        nc.scalar.copy(dst, tp.bitcast(F32))

                for g in range(2):
                    for cb in range(CB):
                        tp_group(g, cb)

                # ------- mirror lower blocks: (cb,db) = (db,cb)^T -------
                for cb in range(CB):
                    for db in range(cb):
                        mir = psum_pool.tile(
                            [128, 128], F32R, tag="ps", name="mir"
                        )
                        nc.tensor.transpose(mir, msrcs[(cb, db)], ident_r)
                        if (cb + db) % 2 == 0:
                            nc.vector.tensor_copy(
                                asb[cb][:, db * 128:(db + 1) * 128],
                                mir.bitcast(F32),
                            )
                        else:
                            nc.scalar.copy(
                                asb[cb][:, db * 128:(db + 1) * 128],
                                mir.bitcast(F32),
                            )

                # ------------- softmax rows + gamma fold -------------
                attn_tiles = []
                for cb in range(CB):
                    negmax = stats_pool.tile([128, 1], F32, tag="st")
                    nc.vector.reduce_max(
                        negmax, asb[cb], axis=mybir.AxisListType.X, negate=True
                    )
                    rowsum = stats_pool.tile([128, 1], F32, tag="st")
                    nc.scalar.activation(
                        asb[cb],
                        asb[cb],
                        mybir.ActivationFunctionType.Exp,
                        bias=negmax,
                        scale=1.0,
                        accum_out=rowsum,
                    )
                    grec = stats_pool.tile([128, 1], F32, tag="st")
                    nc.vector.reciprocal(grec, rowsum)
                    # fold gamma into the row normalizer: attn' = gamma/rowsum * E
                    nc.vector.tensor_scalar_mul(grec, grec, gam)
                    nc.vector.tensor_scalar_mul(asb[cb], asb[cb], grec)
                    # + I on this tile's diagonal block so pass2 fuses the residual
                    nc.vector.tensor_add(
                        asb[cb][:, cb * 128:(cb + 1) * 128],
                        asb[cb][:, cb * 128:(cb + 1) * 128],
                        ident,
                    )
                    # rounded f32r copy for the pass-2 matmul
                    ar = attn_pool.tile([128, C], F32R, tag="attn")
                    nc.vector.tensor_copy(ar, asb[cb])
                    attn_tiles.append(ar)

                # ---------------- rest of a -> aT ----------------
                for g in range(2, NT // 4):
                    for cb in range(CB):
                        tp_group(g, cb)

                # ------------ pass 2: out = a @ (g*attn + I) ------------
                for k in range(NT):
                    yp = psum_pool.tile([128, C], F32, tag="ps")
                    for cb in range(CB):
                        nc.tensor.matmul(
                            yp,
                            at_tiles[cb][:, k * 128:(k + 1) * 128],
                            attn_tiles[cb],
                            start=(cb == 0),
                            stop=(cb == CB - 1),
                        )
                    o = out_pool.tile([128, C], F32, tag="o")
                    if k % 2 == 0:
                        nc.vector.tensor_copy(o, yp)
                    else:
                        nc.scalar.copy(o, yp)
                    nc.sync.dma_start(out=out[b, k * 128:(k + 1) * 128, :], in_=o)

    nc.compile()
    return nc


_NC_CACHE = None


def _get_nc():
    global _NC_CACHE
    if _NC_CACHE is None:
        _NC_CACHE = build_bass()
    return _NC_CACHE


def make_in_maps(x: np.ndarray, gamma: np.ndarray):
    x = np.ascontiguousarray(np.asarray(x, dtype=np.float32)).reshape(B, HW, C)
    gamma = np.ascontiguousarray(np.asarray(gamma, dtype=np.float32)).reshape(1)
    return [
        {"x": x[i * BPC:(i + 1) * BPC], "gamma": gamma} for i in range(NCORES)
    ]


def kernel(x: np.ndarray, gamma: np.ndarray, _trace: bool = False, _tmpdir=None):
    nc = _get_nc()
    in_maps = make_in_maps(x, gamma)
    res = run_bass_kernel_spmd(
        nc, in_maps, list(range(NCORES)), trace=_trace, tmpdir=_tmpdir
    )
    outs = [np.asarray(res.results[i]["out"]) for i in range(NCORES)]
    full = np.concatenate(outs, axis=0).reshape(B, H, W, C)
    if _trace:
        return full, res
    return full


# revision 15
# speedup vs baseline: 1.0462x; 1.0078x over previous
"""Channel-attention module (CAM) kernel for Trainium2.

Reference computation (per batch b):
    a    = x[b].reshape(HW, C)                      # [4096, 512]
    aTa  = a.T @ a                                  # [512, 512]
    attn = softmax(aTa, axis=-1)
    y    = a @ attn                                 # [4096, 512]
    out[b] = gamma * y + x[b]

Sharding: data-parallel over batch B=16 across 8 NeuronCores (2 batches
per core), gamma replicated.  No collectives needed.

Per-core schedule (per batch):
  warmup  ~30 tiny matmuls while the first DMAs land, so the PE HAM clock
          gate is already at 8/8 when real work starts
  pass 1  aTa is symmetric: only diagonal+upper blocks are computed
          (rhs free dim 512/384/256/128 per column-block), lower blocks are
          mirrored via 6 PE transposes of the upper ones
  softmax rows of aTa (SBUF), folding gamma into the normalizer and adding
          the identity so pass 2 directly yields gamma*y + a = a @ (g*attn+I)
  tpose   a -> aT via PE transpose (128x128 blocks, 4 per PSUM bank),
          evacuated to SBUF by Vector/Scalar engines
  pass 2  y[k] (PSUM) += aT[cb][:,k128]-block @ attn'[cb]; copy to SBUF,
          DMA out.

All matmul operands are float32r (fp32 truncated to fp22 by the PE) so the
matmuls run at 1 cycle/row instead of fp32's 4.  The BIR verifier requires
every producer of an f32r matmul operand to emit f32r, so those tiles are
declared float32r and their writers (DMA / DVE / ACT copies) write f32r.
"""

import numpy as np

import concourse.bass as bass
import concourse.bacc as bacc
import concourse.mybir as mybir
import concourse.tile as tile
from concourse.bass_utils import run_bass_kernel_spmd
from concourse.masks import make_identity

B, H, W, C = 16, 64, 64, 512
HW = H * W                      # 4096
NCORES = 8
BPC = B // NCORES               # batches per core
NT = HW // 128                  # 32 row-chunks of a
CB = C // 128                   # 4 column-blocks of C
F32 = mybir.dt.float32
F32R = mybir.dt.float32r


def build_bass():
    nc = bacc.Bacc("TRN2", target_bir_lowering=False, debug=False)
    x = nc.dram_tensor("x", [BPC, HW, C], F32, kind="ExternalInput").ap()
    gamma = nc.dram_tensor("gamma", [1], F32, kind="ExternalInput").ap()
    out = nc.dram_tensor("out", [BPC, HW, C], F32, kind="ExternalOutput").ap()

    with tile.TileContext(nc) as tc:
        with (
            tc.tile_pool(name="singles", bufs=1) as singles,
            tc.tile_pool(name="a", bufs=36) as a_pool,
            tc.tile_pool(name="at", bufs=4) as at_pool,
            tc.tile_pool(name="atasb", bufs=6) as atasb_pool,
            tc.tile_pool(name="attn", bufs=6) as attn_pool,
            tc.tile_pool(name="stats", bufs=16) as stats_pool,
            tc.tile_pool(name="ostage", bufs=6) as out_pool,
            tc.tile_pool(name="psum", bufs=8, space="PSUM") as psum_pool,
        ):
            ident = singles.tile([128, 128], F32)
            make_identity(nc, ident)
            ident_r = singles.tile([128, 128], F32R)
            nc.vector.tensor_copy(ident_r, ident)
            gam = singles.tile([128, 1], F32)
            nc.gpsimd.dma_start(out=gam, in_=gamma.to_broadcast((128, 1)))

            for b in range(BPC):
                # ------- load a + pass 1 (aTa, diag+upper blocks), interleaved -------
                a_tiles = []
                ata = [
                    psum_pool.tile([128, C], F32, tag="ps", name="ata")
                    for _ in range(CB)
                ]
                for k in range(NT):
                    ak = a_pool.tile([128, C], F32R, tag="a", name="a")
                    nc.sync.dma_start(
                        out=ak, in_=x[b, k * 128:(k + 1) * 128, :].bitcast(F32R)
                    )
                    a_tiles.append(ak)
                    for cb in range(CB):
                        nc.tensor.matmul(
                            ata[cb][:, cb * 128:C],
                            ak[:, cb * 128:(cb + 1) * 128],
                            ak[:, cb * 128:C],
                            start=(k == 0),
                            stop=(k == NT - 1),
                        )

                # ------- evacuate upper blocks + stage mirror sources -------
                asb = [
                    atasb_pool.tile([128, C], F32, tag="atasb", name="asb")
                    for _ in range(CB)
                ]
                for cb in range(CB):
                    if cb % 2 == 0:
                        nc.vector.tensor_copy(
                            asb[cb][:, cb * 128:C], ata[cb][:, cb * 128:C]
                        )
                    else:
                        nc.scalar.copy(
                            asb[cb][:, cb * 128:C], ata[cb][:, cb * 128:C]
                        )
                msrcs = {}
                for cb in range(CB):
                    for db in range(cb):
                        # stage upper block (db, cb) as f32r SBUF for the PE
                        msrc = atasb_pool.tile(
                            [128, 128], F32R, tag="msrc", name="msrc", bufs=8
                        )
                        if (cb + db) % 2 == 0:
                            nc.vector.tensor_copy(
                                msrc, ata[db][:, cb * 128:(cb + 1) * 128]
                            )
                        else:
                            nc.scalar.copy(
                                msrc, ata[db][:, cb * 128:(cb + 1) * 128]
                            )
                        msrcs[(cb, db)] = msrc

                # two a->aT transpose groups first: PE keeps busy while the
                # mirror sources are staged by DVE/ACT
                at_tiles = [
                    at_pool.tile([128, HW], F32R, tag="at", name="at")
                    for _ in range(CB)
                ]

                def tp_group(g, cb):
                    tp = psum_pool.tile(
                        [128, C], F32R, tag="ps", name="tp"
                    )
                    for j in range(4):
                        k = g * 4 + j
                        nc.tensor.transpose(
                            tp[:, j * 128:(j + 1) * 128],
                            a_tiles[k][:, cb * 128:(cb + 1) * 128],
                            ident_r,
                        )
                    dst = at_tiles[cb][:, g * 512:(g + 1) * 512]
                    if (g * CB + cb) % 3 == 0:
                        nc.vector.tensor_copy(dst, tp.bitcast(F32))
                    else:
                        nc.scalar.copy(dst, tp.bitcast(F32))

                for g in range(2):
                    for cb in range(CB):
                        tp_group(g, cb)

                # ------- mirror lower blocks: (cb,db) = (db,cb)^T -------
                for cb in range(CB):
                    for db in range(cb):
                        mir = psum_pool.tile(
                            [128, 128], F32R, tag="ps", name="mir"
                        )
                        nc.tensor.transpose(mir, msrcs[(cb, db)], ident_r)
                        if (cb + db) % 2 == 0:
                            nc.vector.tensor_copy(
                                asb[cb][:, db * 128:(db + 1) * 128],
                                mir.bitcast(F32),
                            )
                        else:
                            nc.scalar.copy(
                                asb[cb][:, db * 128:(db + 1) * 128],
                                mir.bitcast(F32),
                            )

                # ------------- softmax rows + gamma fold -------------
                attn_tiles = []
                for cb in range(CB):
                    negmax = stats_pool.tile([128, 1], F32, tag="st")
                    nc.vector.reduce_max(
                        negmax, asb[cb], axis=mybir.AxisListType.X, negate=True
                    )
                    rowsum = stats_pool.tile([128, 1], F32, tag="st")
                    nc.scalar.activation(
                        asb[cb],
                        asb[cb],
                        mybir.ActivationFunctionType.Exp,
                        bias=negmax,
                        scale=1.0,
                        accum_out=rowsum,
                    )
                    grec = stats_pool.tile([128, 1], F32, tag="st")
                    nc.vector.reciprocal(grec, rowsum)
                    # fold gamma into the row normalizer: attn' = gamma/rowsum * E
                    nc.vector.tensor_scalar_mul(grec, grec, gam)
                    nc.vector.tensor_scalar_mul(asb[cb], asb[cb], grec)
                    # + I on this tile's diagonal block so pass2 fuses the residual
                    nc.vector.tensor_add(
                        asb[cb][:, cb * 128:(cb + 1) * 128],
                        asb[cb][:, cb * 128:(cb + 1) * 128],
                        ident,
                    )
                    # rounded f32r copy for the pass-2 matmul
                    ar = attn_pool.tile([128, C], F32R, tag="attn")
                    nc.vector.tensor_copy(ar, asb[cb])
                    attn_tiles.append(ar)

                # ---------------- rest of a -> aT ----------------
                for g in range(2, NT // 4):
                    for cb in range(CB):
                        tp_group(g, cb)

                # ------------ pass 2: out = a @ (g*attn + I) ------------
                for k in range(NT):
                    yp = psum_pool.tile([128, C], F32, tag="ps")
                    for cb in range(CB):
                        nc.tensor.matmul(
                            yp,
                            at_tiles[cb][:, k * 128:(k + 1) * 128],
                            attn_tiles[cb],
                            start=(cb == 0),
                            stop=(cb == CB - 1),
                        )
                    o = out_pool.tile([128, C], F32, tag="o")
                    if k % 2 == 0:
                        nc.vector.tensor_copy(o, yp)
                    else:
                        nc.scalar.copy(o, yp)
                    nc.sync.dma_start(out=out[b, k * 128:(k + 1) * 128, :], in_=o)

    nc.compile()
    return nc


_NC_CACHE = None


def _get_nc():
    global _NC_CACHE
    if _NC_CACHE is None:
        _NC_CACHE = build_bass()
    return _NC_CACHE


def make_in_maps(x: np.ndarray, gamma: np.ndarray):
    x = np.ascontiguousarray(np.asarray(x, dtype=np.float32)).reshape(B, HW, C)
    gamma = np.ascontiguousarray(np.asarray(gamma, dtype=np.float32)).reshape(1)
    return [
        {"x": x[i * BPC:(i + 1) * BPC], "gamma": gamma} for i in range(NCORES)
    ]


def kernel(x: np.ndarray, gamma: np.ndarray, _trace: bool = False, _tmpdir=None):
    nc = _get_nc()
    in_maps = make_in_maps(x, gamma)
    res = run_bass_kernel_spmd(
        nc, in_maps, list(range(NCORES)), trace=_trace, tmpdir=_tmpdir
    )
    outs = [np.asarray(res.results[i]["out"]) for i in range(NCORES)]
    full = np.concatenate(outs, axis=0).reshape(B, H, W, C)
    if _trace:
        return full, res
    return full
